# revision 20
# baseline (speedup 1.0000x reference)
"""Trainium2 Bass kernel for a dense transformer block (B=2,S=2048,D=1024,H=16,DFF=4096).

Sharding across 8 NeuronCores:
  core c: batch b=c//4, group rank r=c%4, replica groups [[0,1,2,3],[4,5,6,7]].
  - Every core loads the FULL x of its batch; LN1 + transpose are replicated
    (no collective needed for the attention input).
  - Attention: head-parallel (4 heads/core, full causal sequence), output
    kept on-chip.
  - out_proj: each core computes the partial sum over its own heads for ALL
    2048 tokens; a single ReduceScatter(add) in bf16 then hands each core its
    own summed 512-token strip (output bytes 1MB vs the 8MB an AllGather of
    the attention output would move).
  - residual, LN2, FFN: token-sharded (512 tokens/core), full weights.
Matmuls run in bf16 (psum accumulation stays fp32); LN statistics and the
residual spine stay fp32 (x is staged twice: bf16 full-batch for LN1,
fp32 own strip for the residual).
Schedule notes: the cost model's DMA device serves transfers by readiness,
so large prefetches are chunked and issue-ordered to never starve the
LN-paced x stream; attention is qc-outer with a software-pipelined kt loop
(scores(kt+1) traced before AV(kt)) and each strip's out_proj runs inside
the Act-bound attention window; W2 prefetch fills the ReduceScatter gap.
"""
import sys

sys.path.insert(0, "/opt/trn_rl_repo")

import numpy as np
import ml_dtypes

import concourse.bass as bass
import concourse.mybir as mybir
import concourse.tile as tile
from concourse import bacc
from concourse.bass_utils import run_bass_kernel_spmd
from concourse.masks import make_identity

AF = mybir.ActivationFunctionType
ALU = mybir.AluOpType
F32 = mybir.dt.float32
F32R = mybir.dt.float32r
BF16 = mybir.dt.bfloat16

B, S, D, H = 2, 2048, 1024, 16
DH = D // H          # 64
DFF = 4 * D          # 4096
EPS = 1e-5
NC = 8               # cores
G = 4                # cores per group (per batch)
TS = S // G          # 512 tokens per strip
HC = H // G          # 4 heads per core
CC = HC * DH         # 256 head-columns per core
P = 128
KD = D // P          # 8 k-tiles over D
KF = DFF // P        # 32 k-tiles over DFF
NTT = S // P         # 16 token tiles (full sequence)
NT = TS // P         # 4 token tiles per strip
GROUPS = [[0, 1, 2, 3], [4, 5, 6, 7]]

_CACHE = {}


def build():
    nc = bacc.Bacc(None)

    io = {}
    io["x_d"] = nc.declare_dram_parameter("x", [S, D], BF16, isOutput=False)
    io["xo_d"] = nc.declare_dram_parameter("xo", [TS, D], F32, isOutput=False)
    io["ln1g_d"] = nc.declare_dram_parameter("ln1_g", [D], F32, isOutput=False)
    io["ln1b_d"] = nc.declare_dram_parameter("ln1_b", [D], F32, isOutput=False)
    io["wq_d"] = nc.declare_dram_parameter("Wq", [D, CC], BF16, isOutput=False)
    io["wk_d"] = nc.declare_dram_parameter("Wk", [D, CC], BF16, isOutput=False)
    io["wv_d"] = nc.declare_dram_parameter("Wv", [D, CC], BF16, isOutput=False)
    io["bq_d"] = nc.declare_dram_parameter("bq", [CC], F32, isOutput=False)
    io["bk_d"] = nc.declare_dram_parameter("bk", [CC], F32, isOutput=False)
    io["bv_d"] = nc.declare_dram_parameter("bv", [CC], F32, isOutput=False)
    io["wo_d"] = nc.declare_dram_parameter("Wo", [CC, D], BF16, isOutput=False)
    io["bo_d"] = nc.declare_dram_parameter("bo", [D], F32, isOutput=False)
    io["ln2g_d"] = nc.declare_dram_parameter("ln2_g", [D], F32, isOutput=False)
    io["ln2b_d"] = nc.declare_dram_parameter("ln2_b", [D], F32, isOutput=False)
    io["w1_d"] = nc.declare_dram_parameter("W1", [D, DFF], BF16, isOutput=False)
    io["b1_d"] = nc.declare_dram_parameter("b1", [DFF], F32, isOutput=False)
    io["w2_d"] = nc.declare_dram_parameter("W2", [DFF, D], BF16, isOutput=False)
    io["b2_d"] = nc.declare_dram_parameter("b2", [D], F32R, isOutput=False)
    io["cmask_d"] = nc.declare_dram_parameter("cmask", [P, 4, 2 * TS], BF16,
                                              isOutput=False)
    io["y_d"] = nc.declare_dram_parameter("y", [TS, D], F32, isOutput=True)

    io["rs_in"] = nc.dram_tensor("rs_in", [S, D], BF16)
    io["rs_out"] = nc.dram_tensor("rs_out", [TS, D], BF16)

    with tile.TileContext(nc) as tc:
        _body(nc, tc, io)
    nc.compile()
    return nc


def _body(nc, tc, t):
    with tc.tile_pool(name="const", bufs=1) as cst:
        # ---- tiny constants + the LN1 params gate phase A: issue first ----
        ln1g = cst.tile([P, KD], F32)
        ln1b = cst.tile([P, KD], F32)
        nc.sync.dma_start(ln1g[:], t["ln1g_d"].rearrange("(k p) -> p k", p=P))
        nc.sync.dma_start(ln1b[:], t["ln1b_d"].rearrange("(k p) -> p k", p=P))

        identb = cst.tile([P, P], BF16)
        make_identity(nc, identb[:])
        epsc = cst.tile([P, 1], F32)
        nc.gpsimd.memset(epsc[:], EPS)
        onesrow_f = cst.tile([1, P], F32)
        nc.gpsimd.memset(onesrow_f[:], 1.0)
        ones128 = cst.tile([1, P], F32R)
        nc.vector.tensor_copy(ones128[:], onesrow_f[:])
        onescol4 = cst.tile([P, HC, 1], F32)
        nc.gpsimd.memset(onescol4[:], 1.0)

        # ---------------- helpers ----------------
        def layernorm(src_tiles, dst_tiles, sc, sq):
            # var = E[x^2] - mu^2; normalize is one fused (x - mu) * inv pass.
            # reduce/normalize alternate between DVE and Pool to halve the
            # DVE serial time; the Square/Sqrt chain stays on Act.
            n = len(src_tiles)
            for mt in range(n):
                ve = nc.vector if mt % 2 == 0 else nc.gpsimd
                xt = src_tiles[mt]
                mu = sc.tile([P, 1], F32, tag="mu", name="mu")
                nc.vector.tensor_reduce(out=mu[:], in_=xt[:], op=ALU.add,
                                        axis=mybir.AxisListType.X)
                mus = sc.tile([P, 1], F32, tag="mus", name="mus")
                nc.scalar.mul(mus[:], mu[:], 1.0 / D)
                sumsq = sc.tile([P, 1], F32, tag="sumsq", name="sumsq")
                nc.scalar.activation(sq[:], xt[:], AF.Square, accum_out=sumsq[:])
                mu2 = sc.tile([P, 1], F32, tag="mu2", name="mu2")
                nc.scalar.activation(mu2[:], mus[:], AF.Square)
                vpe = sc.tile([P, 1], F32, tag="vpe", name="vpe")
                ve.tensor_scalar(out=vpe[:], in0=sumsq[:],
                                 scalar1=1.0 / D, scalar2=mu2[:],
                                 op0=ALU.mult, op1=ALU.subtract)
                std = sc.tile([P, 1], F32, tag="std", name="std")
                nc.scalar.activation(std[:], vpe[:], AF.Sqrt, bias=epsc[:])
                inv = sc.tile([P, 1], F32, tag="inv", name="inv")
                nc.vector.reciprocal(inv[:], std[:])
                nc.vector.tensor_scalar(out=dst_tiles[mt][:], in0=xt[:],
                                        scalar1=mus[:], scalar2=inv[:],
                                        op0=ALU.subtract, op1=ALU.mult)

        def transpose_strip(h_tiles, dst, g_sb, b_sb, tp, k, base_mt, nmt,
                            evict_act):
            # transpose nmt 128x128 blocks of column-tile k into one psum
            # bank, then evict once with the fused *g+b (DVE or Act).
            ps = tp.tile([P, NT * P], BF16, tag="tps", name="tps")
            for j in range(nmt):
                nc.tensor.transpose(ps[:, j * P:(j + 1) * P],
                                    h_tiles[base_mt + j][:, k * P:(k + 1) * P],
                                    identb[:])
            w = nmt * P
            if evict_act:
                nc.scalar.activation(dst[:, :w], ps[:, :w], AF.Identity,
                                     bias=b_sb[:, k:k + 1],
                                     scale=g_sb[:, k:k + 1])
            else:
                nc.vector.tensor_scalar(out=dst[:, :w], in0=ps[:, :w],
                                        scalar1=g_sb[:, k:k + 1],
                                        scalar2=b_sb[:, k:k + 1],
                                        op0=ALU.mult, op1=ALU.add)

        # wo + masks live in the const pool
        wot = cst.tile([P, 2, D], BF16, tag="wot", name="wot")
        wo = [wot[:, kk, :] for kk in range(2)]
        cmaskt = cst.tile([P, 4, 2 * TS], BF16, tag="cmask", name="cmask")

        # ============ persistent pools (LIFO lifetimes) ============
        xop_cm = tc.tile_pool(name="xop", bufs=1)
        xop = xop_cm.__enter__()
        xot = xop.tile([P, NT, D], F32, tag="xot", name="xot")
        xo = [xot[:, mt, :] for mt in range(NT)]

        w1p_cm = tc.tile_pool(name="w1p", bufs=1)
        w1p = w1p_cm.__enter__()
        w1t = w1p.tile([P, KD, DFF], BF16, tag="w1t", name="w1t")
        w1s = [w1t[:, k, :] for k in range(KD)]

        qkvp_cm = tc.tile_pool(name="qkvP", bufs=1)
        qkv = qkvp_cm.__enter__()
        qT = [qkv.tile([P, S], BF16, tag=f"qT{m}", name=f"qT{m}") for m in range(2)]
        kT = [qkv.tile([P, S], BF16, tag=f"kT{m}", name=f"kT{m}") for m in range(2)]
        vo = [qkv.tile([P, HC, DH + 1], BF16, tag=f"vo{tm}", name=f"vo{tm}")
              for tm in range(NTT)]
        aT = [qkv.tile([P, S], BF16, tag=f"aT{kk}", name=f"aT{kk}")
              for kk in range(2)]

        wqkvp_cm = tc.tile_pool(name="wqkv", bufs=1)
        wqkvp = wqkvp_cm.__enter__()
        wqt = wqkvp.tile([P, KD, CC], BF16, tag="wqt", name="wqt")
        wkt = wqkvp.tile([P, KD, CC], BF16, tag="wkt", name="wkt")
        wvt = wqkvp.tile([P, KD, CC], BF16, tag="wvt", name="wvt")
        wq = [wqt[:, k, :] for k in range(KD)]
        wk = [wkt[:, k, :] for k in range(KD)]
        wv = [wvt[:, k, :] for k in range(KD)]

        hTp_cm = tc.tile_pool(name="hTp", bufs=1)
        hTp = hTp_cm.__enter__()
        hT = [hTp.tile([P, S], BF16, tag=f"hT{k}", name=f"hT{k}")
              for k in range(KD)]

        # ============ phase A: x stream + LN1 + transpose ============
        with tc.tile_pool(name="xs", bufs=2) as xsp, \
             tc.tile_pool(name="hs", bufs=1) as hsp, \
             tc.tile_pool(name="sqA", bufs=1) as sqp, \
             tc.tile_pool(name="lnA", bufs=2) as sc, \
             tc.tile_pool(name="tpA", bufs=2, space="PSUM") as tpA:
            sqA = sqp.tile([P, D], F32, tag="sqA", name="sqA")
            for mtg in range(NTT // NT):
                hcur = []
                for j in range(NT):
                    mt = mtg * NT + j
                    xt = xsp.tile([P, D], BF16, tag="x", name="x")
                    nc.sync.dma_start(xt[:], t["x_d"][mt * P:(mt + 1) * P, :])
                    ht = hsp.tile([P, D], BF16, tag=f"h{j}", name=f"h{j}")
                    layernorm([xt], [ht], sc, sqA)
                    hcur.append(ht)
                for k in range(KD):
                    transpose_strip(hcur,
                                    hT[k][:, mtg * NT * P:(mtg + 1) * NT * P],
                                    ln1g, ln1b, tpA, k, 0, NT,
                                    evict_act=(k % 2 == 1))
                if mtg == 0:
                    # QKV weights gate phase B: issue early, they slot into
                    # gaps of the LN-paced x stream (small transfers)
                    nc.sync.dma_start(
                        wqt[:], t["wq_d"].rearrange("(k p) c -> p k c", p=P))
                    nc.sync.dma_start(
                        wkt[:], t["wk_d"].rearrange("(k p) c -> p k c", p=P))
                    nc.sync.dma_start(
                        wvt[:], t["wv_d"].rearrange("(k p) c -> p k c", p=P))

            # needed at attention start / later; chunked so they never hold
            # the readiness-ordered DMA device for long
            nc.sync.dma_start(wot[:], t["wo_d"].rearrange("(k p) d -> p k d", p=P))
            nc.sync.dma_start(cmaskt[:], t["cmask_d"][:])
            for i in range(NT):
                nc.sync.dma_start(xot[:, i, :], t["xo_d"][i * P:(i + 1) * P, :])
            for k in range(KD):
                nc.sync.dma_start(w1t[:, k, :], t["w1_d"][k * P:(k + 1) * P, :])

        # remaining small constants (issued behind x on SP; needed later)
        ln2g = cst.tile([P, KD], F32)
        ln2b = cst.tile([P, KD], F32)
        nc.sync.dma_start(ln2g[:], t["ln2g_d"].rearrange("(k p) -> p k", p=P))
        nc.sync.dma_start(ln2b[:], t["ln2b_d"].rearrange("(k p) -> p k", p=P))
        bqp = cst.tile([P, 2], F32)
        bkp = cst.tile([P, 2], F32)
        nc.sync.dma_start(bqp[:], t["bq_d"].rearrange("(m p) -> p m", p=P))
        nc.sync.dma_start(bkp[:], t["bk_d"].rearrange("(m p) -> p m", p=P))
        bvrow = cst.tile([1, CC], F32)
        nc.sync.dma_start(bvrow[:], t["bv_d"][None, :])
        bvb = cst.tile([P, CC], F32)
        nc.gpsimd.partition_broadcast(bvb[:], bvrow[:])
        borow = cst.tile([1, D], F32)
        nc.sync.dma_start(borow[:], t["bo_d"][None, :])
        bob = cst.tile([P, D], F32)
        nc.gpsimd.partition_broadcast(bob[:], borow[:])
        b1p = cst.tile([P, KF], F32)
        nc.sync.dma_start(b1p[:], t["b1_d"].rearrange("(k p) -> p k", p=P))
        b2r = cst.tile([1, D], F32R)
        nc.sync.dma_start(b2r[:], t["b2_d"][None, :])


        # ============ phase B: QKV projections ============
        with tc.tile_pool(name="projPS", bufs=2, space="PSUM") as pps, \
             tc.tile_pool(name="vPS", bufs=2, space="PSUM") as vps:
            for (w_sb, b_sb, out_sb) in ((wq, bqp, qT), (wk, bkp, kT)):
                for m in range(2):
                    for st in range(G):
                        ps = pps.tile([P, TS], F32, tag="pp", name="pp")
                        for k in range(KD):
                            nc.tensor.matmul(
                                ps[:], w_sb[k][:, m * P:(m + 1) * P],
                                hT[k][:, st * TS:(st + 1) * TS],
                                start=(k == 0), stop=(k == KD - 1))
                        if st % 2 == 0:
                            nc.vector.tensor_scalar(
                                out=out_sb[m][:, st * TS:(st + 1) * TS],
                                in0=ps[:], scalar1=b_sb[:, m:m + 1],
                                scalar2=None, op0=ALU.add)
                        else:
                            nc.scalar.activation(
                                out_sb[m][:, st * TS:(st + 1) * TS], ps[:],
                                AF.Identity, bias=b_sb[:, m:m + 1])
            for tm in range(NTT):
                ps = vps.tile([P, CC], F32, tag="vp", name="vp")
                for k in range(KD):
                    nc.tensor.matmul(
                        ps[:], hT[k][:, tm * P:(tm + 1) * P], wv[k][:],
                        start=(k == 0), stop=(k == KD - 1))
                nc.vector.tensor_tensor(
                    out=vo[tm][:, :, 0:DH],
                    in0=ps[:].rearrange("p (h e) -> p h e", h=HC),
                    in1=bvb[:].rearrange("p (h e) -> p h e", h=HC),
                    op=ALU.add)
                nc.vector.tensor_copy(vo[tm][:, :, DH:DH + 1], onescol4[:])

        hTp_cm.__exit__(None, None, None)
        wqkvp_cm.__exit__(None, None, None)


        # ===== phases C+D: attention (qc-outer) + fused out_proj =====
        # per query strip: both head pairs' attention, then that strip's
        # out_proj partial immediately (fills PE while Act runs exp).
        # kt loop is software-pipelined: scores(kt+1) is traced before AV(kt)
        # so PE isn't idle while Act computes exp(kt).
        with (
            tc.tile_pool(name="scPS", bufs=2, space="PSUM") as scp,
            tc.tile_pool(name="avPS", bufs=1, space="PSUM") as avp,
            tc.tile_pool(name="opPS", bufs=1, space="PSUM") as opp,
            tc.tile_pool(name="attnSB", bufs=3) as asb,
            tc.tile_pool(name="opSB", bufs=3) as osb,
        ):
            for qc in range(G):
                kt_max = 4 * qc + 3
                for hp in range(HC // 2):      # head pairs at PE rows 0/64
                    avs = [avp.tile([DH + 1, TS], F32, tag=f"av{j}",
                                    name=f"av{j}") for j in range(2)]

                    def scores(kt):
                        w0 = P * max(0, kt - 4 * qc)
                        sc_ps = scp.tile([P, 2, TS], F32, tag="scp", name="scp")
                        for j in range(2):
                            o = j * DH
                            nc.tensor.matmul(
                                sc_ps[:, j, w0:],
                                kT[hp][o:o + DH, kt * P:(kt + 1) * P],
                                qT[hp][o:o + DH, qc * TS + w0:(qc + 1) * TS],
                                start=True, stop=True)
                        return sc_ps

                    def expmask(kt, sc_ps):
                        e_r = asb.tile([P, 2, TS], BF16, tag="erp", name="erp")
                        if kt < 4 * qc:
                            v0 = 0      # valid columns start
                            nc.scalar.activation(
                                e_r[:].rearrange("p a b -> p (a b)"),
                                sc_ps[:].rearrange("p a b -> p (a b)"),
                                AF.Exp, scale=0.125)
                        else:
                            # diag block d: cols < 128*d are fully masked --
                            # never compute/read them
                            d = kt - 4 * qc
                            v0 = P * d
                            e_f = asb.tile([P, 2, TS], BF16, tag="efp",
                                           name="efp")
                            nc.scalar.activation(
                                e_f[:, :, v0:], sc_ps[:, :, v0:],
                                AF.Exp, scale=0.125)
                            mdv = cmaskt[:, d, :].rearrange(
                                "p (a b) -> p a b", a=2)
                            nc.vector.tensor_tensor(
                                out=e_r[:, :, v0:], in0=e_f[:, :, v0:],
                                in1=mdv[:, :, v0:], op=ALU.mult)
                        return e_r, v0

                    sc_prev = scores(0)
                    for kt in range(kt_max + 1):
                        e_r, v0 = expmask(kt, sc_prev)
                        if kt < kt_max:
                            sc_prev = scores(kt + 1)
                        for j in range(2):
                            nc.tensor.matmul(avs[j][:, v0:],
                                             vo[kt][:, 2 * hp + j, :],
                                             e_r[:, j, v0:],
                                             start=(kt == 0),
                                             stop=(kt == kt_max))
                    for j in range(2):
                        rec = asb.tile([1, TS], F32, tag=f"rec{j}",
                                       name=f"rec{j}")
                        nc.vector.reciprocal(rec[:], avs[j][DH:DH + 1, :])
                        rb = asb.tile([DH, TS], F32, tag=f"rb{j}",
                                      name=f"rb{j}")
                        nc.gpsimd.partition_broadcast(rb[:], rec[:])
                        nc.vector.tensor_tensor(
                            out=aT[hp][j * DH:(j + 1) * DH,
                                       qc * TS:(qc + 1) * TS],
                            in0=avs[j][0:DH, :], in1=rb[:], op=ALU.mult)

                # out_proj partial for this strip's tokens
                for mt in range(qc * NT, (qc + 1) * NT):
                    ps = opp.tile([P, 2, TS], F32, tag="op", name="op")
                    for n in range(2):
                        for kk in range(2):
                            nc.tensor.matmul(
                                ps[:, n, :], aT[kk][:, mt * P:(mt + 1) * P],
                                wo[kk][:, n * TS:(n + 1) * TS],
                                start=(kk == 0), stop=(kk == 1))
                    ot = osb.tile([P, D], BF16, tag="ot", name="ot")
                    if mt % 2 == 0:
                        nc.vector.tensor_copy(
                            ot[:], ps[:].rearrange("p a b -> p (a b)"))
                    else:
                        nc.scalar.activation(
                            ot[:], ps[:].rearrange("p a b -> p (a b)"), AF.Copy)
                    nc.sync.dma_start(t["rs_in"][mt * P:(mt + 1) * P, :], ot[:])

        qkvp_cm.__exit__(None, None, None)

        # ============ ReduceScatter: sum head-partials, keep own strip ======
        nc.gpsimd.collective_compute(
            "ReduceScatter", ALU.add, ins=[t["rs_in"][:]],
            outs=[t["rs_out"][:]], replica_groups=GROUPS,
        )

        # ============ phase E: residual + LN2 + FFN (token-sharded) ========
        # W2 stream pool opens first (LIFO: closes last); prefetch half of W2
        # on the Act DGE queue so it transfers during the ReduceScatter.
        w2p_cm = tc.tile_pool(name="w2st", bufs=16)
        w2p = w2p_cm.__enter__()
        w2ts = {}
        for k2 in range(KF // 2):
            w2t = w2p.tile([P, D], BF16, tag="w2", name="w2")
            nc.scalar.dma_start(w2t[:], t["w2_d"][k2 * P:(k2 + 1) * P, :])
            w2ts[k2] = w2t

        gTp_cm = tc.tile_pool(name="gTp", bufs=1)
        gtp = gTp_cm.__enter__()
        gT = [gtp.tile([P, TS], BF16, tag=f"gT{mf}", name=f"gT{mf}")
              for mf in range(KF)]
        h2Tp_cm = tc.tile_pool(name="h2Tp", bufs=1)
        h2tp = h2Tp_cm.__enter__()
        h2T = [h2tp.tile([P, TS], BF16, tag=f"h2T{k}", name=f"h2T{k}")
               for k in range(KD)]

        with tc.tile_pool(name="rsb", bufs=1) as rsb, \
             tc.tile_pool(name="lnD", bufs=2) as sc2, \
             tc.tile_pool(name="h2P", bufs=1) as h2sp, \
             tc.tile_pool(name="tpD", bufs=2, space="PSUM") as tpD:
            rstt = rsb.tile([P, NT, D], BF16, tag="rst", name="rst")
            rst = [rstt[:, mt, :] for mt in range(NT)]
            nc.sync.dma_start(rstt[:],
                              t["rs_out"].rearrange("(i p) d -> p i d", p=P))
            for mt in range(NT):
                ve = nc.vector if mt % 2 == 0 else nc.gpsimd
                ve.tensor_tensor(out=xo[mt][:], in0=xo[mt][:], in1=rst[mt][:],
                                 op=ALU.add)
                ve.tensor_tensor(out=xo[mt][:], in0=xo[mt][:], in1=bob[:],
                                 op=ALU.add)
            sqD = h2sp.tile([P, D], F32, tag="sqD", name="sqD")
            h2 = [h2sp.tile([P, D], BF16, tag=f"h2{mt}", name=f"h2{mt}")
                  for mt in range(NT)]
            layernorm(xo, h2, sc2, sqD)
            for k in range(KD):
                transpose_strip(h2, h2T[k][:], ln2g, ln2b, tpD, k, 0, NT,
                                evict_act=(k % 2 == 1))

        # ---- fc1 + gelu ----
        with tc.tile_pool(name="gPS", bufs=4, space="PSUM") as gps:
            for mf in range(KF):
                ps = gps.tile([P, TS], F32, tag="g", name="g")
                for k in range(KD):
                    nc.tensor.matmul(
                        ps[:], w1s[k][:, mf * P:(mf + 1) * P], h2T[k][:],
                        start=(k == 0), stop=(k == KD - 1))
                nc.scalar.activation(gT[mf][:], ps[:], AF.Gelu,
                                     bias=b1p[:, mf:mf + 1])
        h2Tp_cm.__exit__(None, None, None)

        # ---- fc2 + residual ----
        with tc.tile_pool(name="fPS", bufs=1, space="PSUM") as fps, \
             tc.tile_pool(name="ySB", bufs=2) as ysb:
            f_ps = [fps.tile([P, 2, TS], F32, tag=f"f{mt}", name=f"f{mt}")
                    for mt in range(NT)]
            for mt in range(NT):
                for n in range(2):
                    nc.tensor.matmul(
                        f_ps[mt][:, n, :], ones128[:],
                        b2r[:, n * TS:(n + 1) * TS], start=True, stop=False)
            for k2 in range(KF):
                if k2 in w2ts:
                    w2t = w2ts[k2]
                else:
                    w2t = w2p.tile([P, D], BF16, tag="w2", name="w2")
                    nc.scalar.dma_start(w2t[:],
                                        t["w2_d"][k2 * P:(k2 + 1) * P, :])
                for mt in range(NT):
                    for n in range(2):
                        nc.tensor.matmul(
                            f_ps[mt][:, n, :],
                            gT[k2][:, mt * P:(mt + 1) * P],
                            w2t[:, n * TS:(n + 1) * TS],
                            start=False, stop=(k2 == KF - 1))
            for mt in range(NT):
                yt = ysb.tile([P, D], F32, tag="y", name="y")
                nc.vector.tensor_tensor(
                    out=yt[:], in0=f_ps[mt][:].rearrange("p a b -> p (a b)"),
                    in1=xo[mt][:], op=ALU.add)
                nc.sync.dma_start(t["y_d"][mt * P:(mt + 1) * P, :], yt[:])
        gTp_cm.__exit__(None, None, None)
        w2p_cm.__exit__(None, None, None)
        w1p_cm.__exit__(None, None, None)
        xop_cm.__exit__(None, None, None)


def _in_maps(inputs):
    f32 = np.float32
    bf16 = ml_dtypes.bfloat16

    def as_bf16(a):
        return np.ascontiguousarray(np.asarray(a, f32).astype(bf16))

    x = np.asarray(inputs["x"], f32)
    # causal masks for the 4 diagonal sub-blocks: keep where col >= p + 128*d
    cmask = np.zeros((P, 4, 2 * TS), np.float32)
    cols = np.arange(TS)[None, :]
    rows = np.arange(P)[:, None]
    for d in range(4):
        m = (cols >= rows + 128 * d).astype(np.float32)
        cmask[:, d, 0:TS] = m
        cmask[:, d, TS:2 * TS] = m
    cmask = cmask.astype(bf16)
    maps = []
    for c in range(NC):
        b, r = c // G, c % G
        c0 = r * CC
        m = {
            "x": as_bf16(x[b]),
            "xo": np.ascontiguousarray(x[b, r * TS:(r + 1) * TS, :]),
            "ln1_g": np.ascontiguousarray(inputs["ln1_g"], f32),
            "ln1_b": np.ascontiguousarray(inputs["ln1_b"], f32),
            "Wq": as_bf16(np.asarray(inputs["Wq"], f32)[:, c0:c0 + CC]),
            "Wk": as_bf16(np.asarray(inputs["Wk"], f32)[:, c0:c0 + CC]),
            "Wv": as_bf16(np.asarray(inputs["Wv"], f32)[:, c0:c0 + CC]),
            "bq": np.ascontiguousarray(np.asarray(inputs["bq"], f32)[c0:c0 + CC]),
            "bk": np.ascontiguousarray(np.asarray(inputs["bk"], f32)[c0:c0 + CC]),
            "bv": np.ascontiguousarray(np.asarray(inputs["bv"], f32)[c0:c0 + CC]),
            "Wo": as_bf16(np.asarray(inputs["Wo"], f32)[c0:c0 + CC, :]),
            "bo": np.ascontiguousarray(inputs["bo"], f32),
            "ln2_g": np.ascontiguousarray(inputs["ln2_g"], f32),
            "ln2_b": np.ascontiguousarray(inputs["ln2_b"], f32),
            "W1": as_bf16(inputs["W1"]),
            "b1": np.ascontiguousarray(inputs["b1"], f32),
            "W2": as_bf16(inputs["W2"]),
            "b2": np.ascontiguousarray(inputs["b2"], f32),
            "cmask": cmask,
        }
        maps.append(m)
    return maps


def _run(inputs, trace=False):
    if "nc" not in _CACHE:
        _CACHE["nc"] = build()
    nc = _CACHE["nc"]
    maps = _in_maps(inputs)
    res = run_bass_kernel_spmd(nc, maps, list(range(NC)), trace=trace)
    out = np.empty((B, S, D), np.float32)
    for c in range(NC):
        b, r = c // G, c % G
        out[b, r * TS:(r + 1) * TS, :] = res.results[c]["y"]
    return out, res


def kernel(**inputs):
    out, _ = _run(inputs, trace=False)
    return out


if __name__ == "__main__":
    build()
    print("build OK")
